# revision 9
# baseline (speedup 1.0000x reference)
"""Trainium2 Bass kernel for nn_Capsule (dynamic routing, 3 iterations).

Reference computation (per batch b, hidden h, routing dim r=64):
  v0 = squash(mean_r x)                      squash(s) = s * ||s||/(1+||s||)
  for u in (v0, v0+v1):
      w   = softmax_r(x * u)                 (softmax over r, per (b,h))
      s   = sum_r w * x
      v   = squash(s)
  return v2                                  shape [B, H]

Sharding: pure data parallel over batch across 8 NeuronCores.

v3 design (bf16 pipeline, group-interleaved):
  - x is cast to bf16 on the HOST and shipped as bf16 (halves HBM traffic,
    enables DVE 2x mode; adds ~8e-3 rel err, within the 2e-2 gate).
  - Per-core: 256 batches = 2 groups of 128 partitions; free dim = (r, h).
  - Both groups are resident in SBUF and their chunk streams interleave, so
    one group's pipeline-fill latency and serial squash chain hide under the
    other group's compute.
  - Engines: PE = identity-matmul sum_r chains (bf16, 1 col/cyc);
    ACT = exp only (+ Square for the squash norm, same table set);
    DVE = logits/prod muls (bf16 2x) + squash scalar math;
    GpSimd = GPN of RT prod r-slices per chunk.
  - num-matmul emission lags den by one chunk so a slow gpsimd prod can't
    head-of-line-block the PE stream.
"""

import numpy as np

B, R, H = 2048, 64, 512
N_CORES = 8
BPC = B // N_CORES  # batches per core
P = 128             # partitions (batches per group)

# Tunables
RT = 8              # r-slices per x tile == compute chunk (ACT FD = RT*H)
GPN = 3             # prod r-slices per chunk computed on GpSimd (rest DVE)
NR_ITERS = 1        # Newton iterations for rsqrt in squash
EPP_BUFS = 2
PPP_BUFS = 3
LGP_BUFS = 2

_PROGRAM_CACHE = {}


def _build_program(bpc=BPC, reps=1):
    import concourse.tile as tile
    from concourse import bacc, mybir

    f32 = mybir.dt.float32
    bf16 = mybir.dt.bfloat16
    i32 = mybir.dt.int32
    AF = mybir.ActivationFunctionType
    OP = mybir.AluOpType
    AX = mybir.AxisListType

    G = bpc // P        # groups of 128 batches
    NT = R // RT        # x tiles per group

    nc = bacc.Bacc(
        "TRN2",
        target_bir_lowering=False,
        debug=False,
        enable_asserts=False,
    )
    x_d = nc.dram_tensor("x", [bpc, R, H], bf16, kind="ExternalInput").ap()
    id_d = nc.dram_tensor("ident", [P, P], bf16, kind="ExternalInput").ap()
    out_d = nc.dram_tensor("out", [bpc, H], f32, kind="ExternalOutput").ap()

    with tile.TileContext(nc) as tc:
        with (
            tc.tile_pool(name="xp", bufs=G * NT) as xp,
            tc.tile_pool(name="lgp", bufs=LGP_BUFS) as lgp,
            tc.tile_pool(name="epp", bufs=EPP_BUFS) as epp,
            tc.tile_pool(name="ppp", bufs=PPP_BUFS) as ppp,
            tc.tile_pool(name="urp", bufs=4) as urp,
            tc.tile_pool(name="dnp", bufs=2) as dnp,
            tc.tile_pool(name="cst", bufs=1) as cst,
            tc.tile_pool(name="outp", bufs=2) as outp,
            tc.tile_pool(name="psp", bufs=2, space="PSUM") as psp,
            tc.tile_pool(name="psm", bufs=2, space="PSUM") as psm,
        ):
            ident = cst.tile([P, P], bf16)
            nc.sync.dma_start(ident[:], id_d)
            identr = ident[:]
            magic = cst.tile([P, 1], i32)
            nc.vector.memset(magic[:], 0x5F3759DF)

            def gsc_from_nrm(nrm, tag):
                """gsc[p,1] = sn/(1+sn), sn = sqrt(nrm).

                rsqrt via bit-hack seed + NR_ITERS Newton steps (VectorE
                only; ACT's Sqrt lives in a different table set than Exp)."""
                half_i = dnp.tile([P, 1], i32, name=f"hi_{tag}", tag="hi")
                nc.vector.tensor_scalar(
                    half_i[:], nrm.bitcast(i32), 1, None,
                    op0=OP.arith_shift_right,
                )
                y0 = dnp.tile([P, 1], i32, name=f"y0_{tag}", tag="y0")
                nc.vector.scalar_tensor_tensor(
                    y0[:], magic[:], 0, half_i[:],
                    op0=OP.bypass, op1=OP.subtract,
                )
                y = y0[:].bitcast(f32)
                for nr in range(NR_ITERS):
                    t1 = dnp.tile([P, 1], f32, name=f"t1_{tag}_{nr}", tag="t1")
                    nc.vector.tensor_mul(t1[:], y, y)
                    t2 = dnp.tile([P, 1], f32, name=f"t2_{tag}_{nr}", tag="t2")
                    nc.vector.tensor_mul(t2[:], t1[:], nrm)
                    t3 = dnp.tile([P, 1], f32, name=f"t3_{tag}_{nr}", tag="t3")
                    nc.vector.tensor_scalar(
                        t3[:], t2[:], -0.5, 1.5, op0=OP.mult, op1=OP.add
                    )
                    yn = dnp.tile([P, 1], f32, name=f"y_{tag}_{nr}", tag="yn")
                    nc.vector.tensor_mul(yn[:], y, t3[:])
                    y = yn[:]
                # y ~ 1/sn ;  gsc = 1/(1 + y)
                y1 = dnp.tile([P, 1], f32, name=f"y1_{tag}", tag="y1")
                nc.vector.tensor_scalar_add(y1[:], y, 1.0)
                gsc = dnp.tile([P, 1], f32, name=f"gsc_{tag}", tag="gsc")
                nc.vector.reciprocal_approx_fast(out=gsc[:], in_=y1[:])
                return gsc

            def norm_of(s_ap, tag):
                """nrm[p,1] = sum_h s^2 — Square on ACT (same table set as
                Exp), row-reduce on DVE."""
                sq = dnp.tile([P, H], f32, name=f"sq_{tag}", tag="sq")
                nc.scalar.activation(sq[:], s_ap, AF.Square)
                nrm = dnp.tile([P, 1], f32, name=f"nrm_{tag}", tag="nrm")
                nc.vector.reduce_sum(nrm[:], sq[:], axis=AX.X)
                return nrm[:]

            def urep_of(u_bf):
                """Broadcast view [P, RT, H] of a bf16 [P, H] u tile; the
                innermost dim keeps step 1 so DVE 2x mode is preserved."""
                return (
                    u_bf[:]
                    .rearrange("p (a h) -> p a h", a=1)
                    .broadcast_to([P, RT, H])
                )

            for rep in range(reps):
                xt = {}   # (g, t) -> tile
                u_bf = {}
                ur = {}
                # ---- load + iter 0 (mean over r) per group; squash0 of
                # group g overlaps the mean matmuls of group g+1
                for g in range(G):
                    xg = x_d[g * P:(g + 1) * P]  # [128, R, H]
                    for t in range(NT):
                        x_t = xp.tile([P, RT, H], bf16, name="xtile", tag="xtile")
                        nc.sync.dma_start(x_t[:], xg[:, t * RT:(t + 1) * RT, :])
                        xt[(g, t)] = x_t
                for g in range(G):
                    mean_ps = psm.tile([P, H], f32, name=f"mean_{g}", tag="mean")
                    for t in range(NT):
                        for r in range(RT):
                            nc.tensor.matmul(
                                mean_ps[:],
                                identr,
                                xt[(g, t)][:, r, :],
                                start=(t == 0 and r == 0),
                                stop=(t == NT - 1 and r == RT - 1),
                            )
                    s0 = dnp.tile([P, H], f32, name=f"s0_{g}", tag="s")
                    nc.vector.tensor_scalar_mul(s0[:], mean_ps[:], 1.0 / R)
                    gsc0 = gsc_from_nrm(norm_of(s0[:], f"{g}_0"), f"{g}_0")
                    ub = urp.tile([P, H], bf16, name=f"u_{g}", tag="ub")
                    nc.vector.tensor_scalar_mul(ub[:], s0[:], gsc0[:])
                    u_bf[g] = ub
                    ur[g] = urep_of(ub)

                # ---- iters 1, 2: both groups' chunk streams interleaved
                for it in (1, 2):
                    den_ps, num_ps, pend = {}, {}, {}
                    for g in range(G):
                        den_ps[g] = psp.tile(
                            [P, H], f32, name=f"den_{g}_{it}", tag="den"
                        )
                        num_ps[g] = psp.tile(
                            [P, H], f32, name=f"num_{g}_{it}", tag="num"
                        )
                        pend[g] = None

                    def emit_num(g, pendg):
                        c, pp_c = pendg
                        for r in range(RT):
                            nc.tensor.matmul(
                                num_ps[g][:],
                                identr,
                                pp_c[:, r, :],
                                start=(c == 0 and r == 0),
                                stop=(c == NT - 1 and r == RT - 1),
                            )

                    for cc in range(G * NT):
                        g, c = cc % G, cc // G
                        # (a) logits = x * u  (bf16, DVE 2x)
                        lg = lgp.tile([P, RT, H], bf16, name="lg", tag="lg")
                        nc.vector.tensor_mul(lg[:], xt[(g, c)][:], ur[g])
                        # (b) e = exp(logits)  (ACT, FD = RT*H)
                        ep = epp.tile([P, RT, H], bf16, name="ep", tag="ep")
                        nc.scalar.activation(ep[:], lg[:], AF.Exp)
                        # (e') pending num matmuls first: their prod input is
                        # already ready, so PE isn't blocked behind den's
                        # wait on exp(c)
                        if pend[g] is not None:
                            emit_num(g, pend[g])
                            pend[g] = None
                        # (c) denom += sum_r e   (PE)
                        for r in range(RT):
                            nc.tensor.matmul(
                                den_ps[g][:],
                                identr,
                                ep[:, r, :],
                                start=(c == 0 and r == 0),
                                stop=(c == NT - 1 and r == RT - 1),
                            )
                        # (d) prod = e * x  (split GpSimd / DVE by r-slices)
                        pp = ppp.tile([P, RT, H], bf16, name="pp", tag="pp")
                        if GPN > 0:
                            nc.gpsimd.tensor_tensor(
                                pp[:, 0:GPN, :], ep[:, 0:GPN, :],
                                xt[(g, c)][:, 0:GPN, :], op=OP.mult,
                            )
                        if GPN < RT:
                            nc.vector.tensor_tensor(
                                pp[:, GPN:RT, :], ep[:, GPN:RT, :],
                                xt[(g, c)][:, GPN:RT, :], op=OP.mult,
                            )
                        # (e) num += sum_r e*x  (PE, lagged one chunk)
                        pend[g] = (c, pp)
                    for g in range(G):
                        emit_num(g, pend[g])

                    # ---- normalize + squash, groups interleaved
                    rd, s = {}, {}
                    for g in range(G):
                        rd[g] = dnp.tile([P, H], f32, name=f"rd_{g}_{it}", tag="rd")
                        nc.vector.reciprocal_approx_fast(
                            out=rd[g][:], in_=den_ps[g][:]
                        )
                    for g in range(G):
                        s[g] = dnp.tile([P, H], f32, name=f"s_{g}_{it}", tag="s")
                        nc.vector.tensor_mul(s[g][:], num_ps[g][:], rd[g][:])
                    nrm = {g: norm_of(s[g][:], f"{g}_{it}") for g in range(G)}
                    gsc = {g: gsc_from_nrm(nrm[g], f"{g}_{it}") for g in range(G)}
                    for g in range(G):
                        if it == 1:
                            u2 = urp.tile([P, H], bf16, name=f"u2_{g}", tag="ub")
                            # u2 = s*gsc + u  (bf16 out; in1 is the bf16 u0)
                            nc.vector.scalar_tensor_tensor(
                                u2[:], s[g][:], gsc[g][:], u_bf[g][:],
                                op0=OP.mult, op1=OP.add,
                            )
                            u_bf[g] = u2
                            ur[g] = urep_of(u2)
                        else:
                            o = outp.tile([P, H], f32, name="o", tag="o")
                            nc.vector.tensor_scalar_mul(o[:], s[g][:], gsc[g][:])
                            nc.sync.dma_start(out_d[g * P:(g + 1) * P, :], o[:])

    nc.compile()
    return nc


def _get_program(bpc=BPC, reps=1):
    key = (bpc, reps, RT, GPN)
    if key not in _PROGRAM_CACHE:
        _PROGRAM_CACHE[key] = _build_program(bpc, reps)
    return _PROGRAM_CACHE[key]


def _make_in_maps(x):
    import ml_dtypes

    xb = np.asarray(x, dtype=np.float32).astype(ml_dtypes.bfloat16)
    shards = xb.reshape(N_CORES, BPC, R, H)
    ident = np.eye(P, dtype=np.float32).astype(ml_dtypes.bfloat16)
    return [
        {"x": np.ascontiguousarray(shards[i]), "ident": ident}
        for i in range(N_CORES)
    ]


def kernel(input_matrix: np.ndarray) -> np.ndarray:
    from concourse.bass_utils import run_bass_kernel_spmd

    x = np.asarray(input_matrix, dtype=np.float32)
    assert x.shape == (B, R, H)
    nc = _get_program()
    in_maps = _make_in_maps(x)
    res = run_bass_kernel_spmd(nc, in_maps, core_ids=list(range(N_CORES)))
    out = np.concatenate(
        [res.results[i]["out"] for i in range(N_CORES)], axis=0
    )
    return out


if __name__ == "__main__":
    nc = _get_program()
    print("program built and compiled OK")


# revision 15
# speedup vs baseline: 1.6018x; 1.6018x over previous
"""Trainium2 Bass kernel for nn_Capsule (dynamic routing, 3 iterations).

Reference computation (per batch b, hidden h, routing dim r=64):
  v0 = squash(mean_r x)                      squash(s) = s * ||s||/(1+||s||)
  for u in (v0, v0+v1):
      w   = softmax_r(x * u)                 (softmax over r, per (b,h))
      s   = sum_r w * x
      v   = squash(s)
  return v2                                  shape [B, H]

Sharding: pure data parallel over batch across 8 NeuronCores.

v3 design (bf16 pipeline, group-interleaved):
  - x is cast to bf16 on the HOST and shipped as bf16 (halves HBM traffic,
    enables DVE 2x mode; adds ~8e-3 rel err, within the 2e-2 gate).
  - Per-core: 256 batches = 2 groups of 128 partitions; free dim = (r, h).
  - Both groups are resident in SBUF and their chunk streams interleave, so
    one group's pipeline-fill latency and serial squash chain hide under the
    other group's compute.
  - Engines: PE = identity-matmul sum_r chains (bf16, 1 col/cyc, FD=512 —
    a single matmul output must stay inside one PSUM bank);
    ACT = exp only (+ Square for the squash norm, same table set);
    DVE = logits/prod muls (bf16 2x) + squash scalar math;
    GpSimd = GPN of RT prod r-slices per chunk.
  - Pending num-matmuls are emitted before the next den block so PE has
    ready work while ACT finishes the next chunk's exp.
"""

import numpy as np

B, R, H = 2048, 64, 512
N_CORES = 8
BPC = B // N_CORES  # batches per core
P = 128             # partitions (batches per group)

# Tunables
RT = 8              # r-slices per x tile == compute chunk (ACT FD = RT*H)
GPN = 3             # prod r-slices per chunk computed on GpSimd (rest DVE)
NR_ITERS = 1        # Newton iterations for rsqrt in squash
EPP_BUFS = 2
PPP_BUFS = 3
LGP_BUFS = 2

_PROGRAM_CACHE = {}


def _build_program(bpc=BPC, reps=1):
    import concourse.tile as tile
    from concourse import bacc, mybir

    f32 = mybir.dt.float32
    bf16 = mybir.dt.bfloat16
    i32 = mybir.dt.int32
    AF = mybir.ActivationFunctionType
    OP = mybir.AluOpType
    AX = mybir.AxisListType

    G = bpc // P        # groups of 128 batches
    NT = R // RT        # x tiles per group

    nc = bacc.Bacc(
        "TRN2",
        target_bir_lowering=False,
        debug=False,
        enable_asserts=False,
    )
    x_d = nc.dram_tensor("x", [bpc, R, H], bf16, kind="ExternalInput").ap()
    id_d = nc.dram_tensor("ident", [P, P], bf16, kind="ExternalInput").ap()
    out_d = nc.dram_tensor("out", [bpc, H], f32, kind="ExternalOutput").ap()

    with tile.TileContext(nc) as tc:
        with (
            tc.tile_pool(name="xp", bufs=G * NT) as xp,
            tc.tile_pool(name="lgp", bufs=LGP_BUFS) as lgp,
            tc.tile_pool(name="epp", bufs=EPP_BUFS) as epp,
            tc.tile_pool(name="ppp", bufs=PPP_BUFS) as ppp,
            tc.tile_pool(name="urp", bufs=4) as urp,
            tc.tile_pool(name="dnp", bufs=2) as dnp,
            tc.tile_pool(name="cst", bufs=1) as cst,
            tc.tile_pool(name="outp", bufs=2) as outp,
            tc.tile_pool(name="psp", bufs=2, space="PSUM") as psp,
            tc.tile_pool(name="psm", bufs=2, space="PSUM") as psm,
        ):
            ident = cst.tile([P, P], bf16)
            nc.sync.dma_start(ident[:], id_d)
            identr = ident[:]
            magic = cst.tile([P, 1], i32)
            nc.vector.memset(magic[:], 0x5F3759DF)

            def gsc_from_nrm(nrm, tag):
                """gsc[p,1] = sn/(1+sn), sn = sqrt(nrm).

                rsqrt via bit-hack seed + NR_ITERS Newton steps (VectorE
                only; ACT's Sqrt lives in a different table set than Exp)."""
                half_i = dnp.tile([P, 1], i32, name=f"hi_{tag}", tag="hi")
                nc.vector.tensor_scalar(
                    half_i[:], nrm.bitcast(i32), 1, None,
                    op0=OP.arith_shift_right,
                )
                y0 = dnp.tile([P, 1], i32, name=f"y0_{tag}", tag="y0")
                nc.vector.scalar_tensor_tensor(
                    y0[:], magic[:], 0, half_i[:],
                    op0=OP.bypass, op1=OP.subtract,
                )
                y = y0[:].bitcast(f32)
                for nr in range(NR_ITERS):
                    t1 = dnp.tile([P, 1], f32, name=f"t1_{tag}_{nr}", tag="t1")
                    nc.vector.tensor_mul(t1[:], y, y)
                    t2 = dnp.tile([P, 1], f32, name=f"t2_{tag}_{nr}", tag="t2")
                    nc.vector.tensor_mul(t2[:], t1[:], nrm)
                    t3 = dnp.tile([P, 1], f32, name=f"t3_{tag}_{nr}", tag="t3")
                    nc.vector.tensor_scalar(
                        t3[:], t2[:], -0.5, 1.5, op0=OP.mult, op1=OP.add
                    )
                    yn = dnp.tile([P, 1], f32, name=f"y_{tag}_{nr}", tag="yn")
                    nc.vector.tensor_mul(yn[:], y, t3[:])
                    y = yn[:]
                # y ~ 1/sn ;  gsc = 1/(1 + y)
                y1 = dnp.tile([P, 1], f32, name=f"y1_{tag}", tag="y1")
                nc.vector.tensor_scalar_add(y1[:], y, 1.0)
                gsc = dnp.tile([P, 1], f32, name=f"gsc_{tag}", tag="gsc")
                nc.vector.reciprocal_approx_fast(out=gsc[:], in_=y1[:])
                return gsc

            def norm_of(s_ap, tag):
                """nrm[p,1] = sum_h s^2 — Square on ACT (same table set as
                Exp), row-reduce on DVE."""
                sq = dnp.tile([P, H], f32, name=f"sq_{tag}", tag="sq")
                nc.scalar.activation(sq[:], s_ap, AF.Square)
                nrm = dnp.tile([P, 1], f32, name=f"nrm_{tag}", tag="nrm")
                nc.vector.reduce_sum(nrm[:], sq[:], axis=AX.X)
                return nrm[:]

            def urep_of(u_bf):
                """Broadcast view [P, RT, H] of a bf16 [P, H] u tile; the
                innermost dim keeps step 1 so DVE 2x mode is preserved."""
                return (
                    u_bf[:]
                    .rearrange("p (a h) -> p a h", a=1)
                    .broadcast_to([P, RT, H])
                )

            for rep in range(reps):
                xt = {}   # (g, t) -> tile
                u_bf = {}
                ur = {}
                # ---- load + iter 0 (mean over r) per group; squash0 of
                # group g overlaps the mean matmuls of group g+1
                for g in range(G):
                    xg = x_d[g * P:(g + 1) * P]  # [128, R, H]
                    for t in range(NT):
                        x_t = xp.tile([P, RT, H], bf16, name="xtile", tag="xtile")
                        nc.sync.dma_start(x_t[:], xg[:, t * RT:(t + 1) * RT, :])
                        xt[(g, t)] = x_t
                for g in range(G):
                    mean_ps = psm.tile([P, H], f32, name=f"mean_{g}", tag="mean")
                    for t in range(NT):
                        for r in range(RT):
                            nc.tensor.matmul(
                                mean_ps[:],
                                identr,
                                xt[(g, t)][:, r, :],
                                start=(t == 0 and r == 0),
                                stop=(t == NT - 1 and r == RT - 1),
                            )
                    s0 = dnp.tile([P, H], f32, name=f"s0_{g}", tag="s")
                    nc.vector.tensor_scalar_mul(s0[:], mean_ps[:], 1.0 / R)
                    gsc0 = gsc_from_nrm(norm_of(s0[:], f"{g}_0"), f"{g}_0")
                    ub = urp.tile([P, H], bf16, name=f"u_{g}", tag="ub")
                    nc.vector.tensor_scalar_mul(ub[:], s0[:], gsc0[:])
                    u_bf[g] = ub
                    ur[g] = urep_of(ub)

                # ---- iters 1, 2: both groups' chunk streams interleaved
                for it in (1, 2):
                    den_ps, num_ps, pend = {}, {}, {}
                    for g in range(G):
                        den_ps[g] = psp.tile(
                            [P, H], f32, name=f"den_{g}_{it}", tag="den"
                        )
                        num_ps[g] = psp.tile(
                            [P, H], f32, name=f"num_{g}_{it}", tag="num"
                        )
                        pend[g] = None

                    def emit_num(g, pendg):
                        c, pp_c = pendg
                        for r in range(RT):
                            nc.tensor.matmul(
                                num_ps[g][:],
                                identr,
                                pp_c[:, r, :],
                                start=(c == 0 and r == 0),
                                stop=(c == NT - 1 and r == RT - 1),
                            )

                    for cc in range(G * NT):
                        g, c = cc % G, cc // G
                        # (a) logits = x * u  (bf16, DVE 2x)
                        lg = lgp.tile([P, RT, H], bf16, name="lg", tag="lg")
                        nc.vector.tensor_mul(lg[:], xt[(g, c)][:], ur[g])
                        # (b) e = exp(logits)  (ACT, FD = RT*H)
                        ep = epp.tile([P, RT, H], bf16, name="ep", tag="ep")
                        nc.scalar.activation(ep[:], lg[:], AF.Exp)
                        # (e') pending num matmuls first: their prod input is
                        # already ready, so PE isn't blocked behind den's
                        # wait on exp(c)
                        if pend[g] is not None:
                            emit_num(g, pend[g])
                            pend[g] = None
                        # (c) denom += sum_r e   (PE)
                        for r in range(RT):
                            nc.tensor.matmul(
                                den_ps[g][:],
                                identr,
                                ep[:, r, :],
                                start=(c == 0 and r == 0),
                                stop=(c == NT - 1 and r == RT - 1),
                            )
                        # (d) prod = e * x  (split GpSimd / DVE by r-slices)
                        pp = ppp.tile([P, RT, H], bf16, name="pp", tag="pp")
                        if GPN > 0:
                            nc.gpsimd.tensor_tensor(
                                pp[:, 0:GPN, :], ep[:, 0:GPN, :],
                                xt[(g, c)][:, 0:GPN, :], op=OP.mult,
                            )
                        if GPN < RT:
                            nc.vector.tensor_tensor(
                                pp[:, GPN:RT, :], ep[:, GPN:RT, :],
                                xt[(g, c)][:, GPN:RT, :], op=OP.mult,
                            )
                        # (e) num += sum_r e*x  (PE, lagged one chunk)
                        pend[g] = (c, pp)
                    for g in range(G):
                        emit_num(g, pend[g])

                    # ---- normalize + squash, groups interleaved
                    rd, s = {}, {}
                    for g in range(G):
                        rd[g] = dnp.tile([P, H], f32, name=f"rd_{g}_{it}", tag="rd")
                        nc.vector.reciprocal_approx_fast(
                            out=rd[g][:], in_=den_ps[g][:]
                        )
                    for g in range(G):
                        s[g] = dnp.tile([P, H], f32, name=f"s_{g}_{it}", tag="s")
                        nc.vector.tensor_mul(s[g][:], num_ps[g][:], rd[g][:])
                    nrm = {g: norm_of(s[g][:], f"{g}_{it}") for g in range(G)}
                    gsc = {g: gsc_from_nrm(nrm[g], f"{g}_{it}") for g in range(G)}
                    for g in range(G):
                        if it == 1:
                            u2 = urp.tile([P, H], bf16, name=f"u2_{g}", tag="ub")
                            # u2 = s*gsc + u  (bf16 out; in1 is the bf16 u0)
                            nc.vector.scalar_tensor_tensor(
                                u2[:], s[g][:], gsc[g][:], u_bf[g][:],
                                op0=OP.mult, op1=OP.add,
                            )
                            u_bf[g] = u2
                            ur[g] = urep_of(u2)
                        else:
                            o = outp.tile([P, H], f32, name="o", tag="o")
                            nc.vector.tensor_scalar_mul(o[:], s[g][:], gsc[g][:])
                            nc.sync.dma_start(out_d[g * P:(g + 1) * P, :], o[:])

    nc.compile()
    return nc


def _get_program(bpc=BPC, reps=1):
    key = (bpc, reps, RT, GPN)
    if key not in _PROGRAM_CACHE:
        _PROGRAM_CACHE[key] = _build_program(bpc, reps)
    return _PROGRAM_CACHE[key]


def _make_in_maps(x):
    import ml_dtypes

    xb = np.asarray(x, dtype=np.float32).astype(ml_dtypes.bfloat16)
    shards = xb.reshape(N_CORES, BPC, R, H)
    ident = np.eye(P, dtype=np.float32).astype(ml_dtypes.bfloat16)
    return [
        {"x": np.ascontiguousarray(shards[i]), "ident": ident}
        for i in range(N_CORES)
    ]


def kernel(input_matrix: np.ndarray) -> np.ndarray:
    from concourse.bass_utils import run_bass_kernel_spmd

    x = np.asarray(input_matrix, dtype=np.float32)
    assert x.shape == (B, R, H)
    nc = _get_program()
    in_maps = _make_in_maps(x)
    res = run_bass_kernel_spmd(nc, in_maps, core_ids=list(range(N_CORES)))
    out = np.concatenate(
        [res.results[i]["out"] for i in range(N_CORES)], axis=0
    )
    return out


if __name__ == "__main__":
    nc = _get_program()
    print("program built and compiled OK")
